# revision 35
# baseline (speedup 1.0000x reference)
# Deformable conv (B=4, C=256, 56x56, 3x3, COUT=256) on 8 Trainium2 cores.
#
# Sharding: core = b*2 + half; each core handles batch b, output rows
# [half*28, half*28+28). Data path in fp16; accumulation in fp32 PSUM;
# offsets/bilinear weights computed in fp32 on the DVE.
#
# Per-core pipeline (~200us/iter on HW):
#   C. offset conv as 9-tap implicit GEMM (fp16) -> offsets [18, 1664]
#   D. PE-transpose offsets to [128p, 13t, 18]; floor/frac via the 1.5*2^23
#      magic-round; border algebra folded into 4 bilinear slot weights
#      W4 [128p, 13t, 4slot, 9k]; ONE pair-gather index per (pos, tap):
#      idx = floor(yb/2)*56 + xb + (yb odd)*1568 into the pair-major table
#      (int16, 16-partition wrapped + tree-replicated for SWDGE)
#   E. per 128-position tile: 2 SWDGE dma_gathers (512/640 idx, on separate
#      SWDGE queues) of 2KB elems from the pair-major x^T copy in DRAM --
#      one elem = all 4 bilinear corners (rows yb,yb+1 x cols xb,xb+1) of a
#      tap; DVE computes only the 4 slot products [128p, 9k, 256c]; the
#      bilinear add tree runs on the PE as 4 PSUM-accumulated transpose
#      matmuls per 128-chunk (fp32 adds for free), ACT evicts PSUM->SBUF
#      f16; per 4 tiles: implicit GEMM over 18 chunks of 128 -> out
#      [256, 512] fp32 -> DRAM.
#
# Host packs: xT3 pair-major table (A copy = row pairs (2Y,2Y+1), B copy =
# (2Y+1,2Y+2)) so any clipped pair start yb in [0,54] is one 2KB elem;
# 3 SWDGE queues + 48KB dynamic DMA scratch keep gather drains overlapped.
import numpy as np
from contextlib import ExitStack

import concourse.bass as bass
import concourse.tile as tile
from concourse import bacc, mybir
from concourse.bass_types import AP
from concourse.bass_utils import run_bass_kernel_spmd

F32 = mybir.dt.float32
F16 = mybir.dt.float16
I16 = mybir.dt.int16
OP = mybir.AluOpType

B, CIN, H, W = 4, 256, 56, 56
COUT, KK = 256, 9
HWp = H * W            # 3136
NPOS = 1664            # 13 * 128 padded positions per core
T = 13                 # position tiles
ROWS_HALF = 28
CONV_ROWS = 32         # host-padded y window rows for conv input
PADW = 58              # x-padded width
CONV_FREE = CONV_ROWS * PADW  # 1856


def build_program(reps: int = 1, debug: bool = False, stop_after: int = 99,
                  skip: frozenset = frozenset(), one_gather: bool = False,
                  n_queues: int = 3, sbuf_gather: bool = False,
                  dma_scratch: int = 49152):
    nc = bacc.Bacc("TRN2", target_bir_lowering=False, debug=False, num_devices=8,
                   num_swdge_queues=n_queues,
                   dynamic_dma_scratch_size=dma_scratch)

    # ---- I/O -------------------------------------------------------------
    # xT: position-major fp16 x (pre-transposed on host), rows HWp..HWp+127 zero
    xT_t = nc.dram_tensor("xT", [HWp + 128, 256], F16, kind="ExternalInput")
    # pair-major x^T: row v = 512 f16 = [ch256 @ row 2Y+par, ch256 @ row 2Y+1+par]
    # A-copy (even pair starts) rows 0..1567, B-copy (odd starts) 1568..3079
    xT3_t = nc.dram_tensor("xT3", [3200, 512], F16, kind="ExternalInput")
    # x16p: host-padded fp16 conv input window [ch, 128, rows, PADW]
    xp_t = nc.dram_tensor("x16p", [2, 128, CONV_ROWS, PADW], F16, kind="ExternalInput")
    wT_t = nc.dram_tensor("wT", [128, 18, 256], F16, kind="ExternalInput")
    ow_t = nc.dram_tensor("offwT", [128, KK, 2, 18], F16, kind="ExternalInput")
    ob_t = nc.dram_tensor("offb", [18, 1], F32, kind="ExternalInput")
    by_t = nc.dram_tensor("base_y", [128, T, KK], F32, kind="ExternalInput")
    bx_t = nc.dram_tensor("base_x", [128, T, KK], F32, kind="ExternalInput")
    id16_t = nc.dram_tensor("ident16", [128, 128], F16, kind="ExternalInput")
    ones1_t = nc.dram_tensor("ones1", [1, 128], F16, kind="ExternalInput")
    id32_t = nc.dram_tensor("ident32", [18, 18], F32, kind="ExternalInput")
    out_t = nc.dram_tensor("out", [2, 128, NPOS], F32, kind="ExternalOutput")
    dbg = {}
    if debug:
        dbg["conv"] = nc.dram_tensor("dbg_conv", [18, NPOS], F32, kind="ExternalOutput")
        dbg["w4"] = nc.dram_tensor("dbg_w4", [128, T, 4, KK], F32, kind="ExternalOutput")
        dbg["wrapped"] = nc.dram_tensor("dbg_wrapped", [128, T, KK, 8], I16, kind="ExternalOutput")
        dbg["g0"] = nc.dram_tensor("dbg_g0", [128, KK, 1024], F16, kind="ExternalOutput")
        dbg["samp0"] = nc.dram_tensor("dbg_samp0", [128, KK, 256], F16, kind="ExternalOutput")
        dbg["xT"] = nc.dram_tensor("dbg_xT", [HWp + 1, 256], F16, kind="ExternalOutput")
        dbg["xg0"] = nc.dram_tensor("dbg_xg0", [128, 4, 18, 128], F16, kind="ExternalOutput")
        dbg["w4r0"] = nc.dram_tensor("dbg_w4r0", [128, 36, 128], F16, kind="ExternalOutput")
        dbg["xt0"] = nc.dram_tensor("dbg_xt0", [128, 18, 512], F16, kind="ExternalOutput")
        dbg["w4ts0"] = nc.dram_tensor("dbg_w4ts0", [36, 128], F16, kind="ExternalOutput")
        dbg["w4flat0"] = nc.dram_tensor("dbg_w4flat0", [1, 36 * 128], F16, kind="ExternalOutput")

    with tile.TileContext(nc) as tc, ExitStack() as ctx:
        # ---- persistent pools -------------------------------------------
        cpool = ctx.enter_context(tc.tile_pool(name="consts", bufs=1))
        wT = cpool.tile([128, 18, 256], F16)
        nc.sync.dma_start(wT[:], wT_t.ap())
        offw = cpool.tile([128, KK, 2, 18], F16)
        nc.sync.dma_start(offw[:], ow_t.ap())
        offb = cpool.tile([18, 1], F32)
        nc.sync.dma_start(offb[:], ob_t.ap())
        base_y = cpool.tile([128, T, KK], F32)
        nc.sync.dma_start(base_y[:], by_t.ap())
        base_x = cpool.tile([128, T, KK], F32)
        nc.sync.dma_start(base_x[:], bx_t.ap())
        id16 = cpool.tile([128, 128], F16)
        nc.sync.dma_start(id16[:], id16_t.ap())
        id32 = cpool.tile([18, 18], F32)
        nc.sync.dma_start(id32[:], id32_t.ap())
        x16p = cpool.tile([128, 2, CONV_ROWS, PADW], F16)
        nc.sync.dma_start(x16p[:, 0], xp_t.ap()[0])
        nc.sync.dma_start(x16p[:, 1], xp_t.ap()[1])
        ones1 = cpool.tile([1, 128], F16)
        nc.sync.dma_start(ones1[:], ones1_t.ap())
        xT_sb = None
        if sbuf_gather:
            _load_xt_sb = True
        else:
            _load_xt_sb = False
        # SBUF-resident x^T for SBUF-source transposing gather:
        # partition p<64 rank r holds xT row r*64+p; partition 64+p holds
        # row r*64+p+1 (the +1 shifted copy), so one 1024B gather elem =
        # rows (v, v+1) via stripe-major addressing (tokens_per_rank=64).
        if _load_xt_sb:
            xT_sb = cpool.tile([128, 51, 256], F16)
            xT_ap = xT_t.ap()
            nc.sync.dma_start(
                xT_sb[0:64, :, :],
                AP(tensor=xT_ap.tensor, offset=xT_ap.offset,
                   ap=[[256, 64], [16384, 51], [1, 256]]))
            nc.sync.dma_start(
                xT_sb[64:128, 0:50, :],
                AP(tensor=xT_ap.tensor, offset=xT_ap.offset + 256,
                   ap=[[256, 64], [16384, 50], [1, 256]]))

        for _rep in range(reps):
            _one_pass(nc, tc, dict(
                wT=wT, offw=offw, offb=offb, base_y=base_y, base_x=base_x,
                id16=id16, id32=id32, x16p=x16p, xT_t=xT_t, xT3_t=xT3_t,
                xT_sb=xT_sb,
                ones1=ones1,
                out_t=out_t, dbg=dbg), stop_after=stop_after, skip=skip,
                one_gather=one_gather, n_queues=n_queues,
                sbuf_gather=sbuf_gather)

    nc.compile()
    return nc


def _one_pass(nc, tc, s, stop_after=99, skip=frozenset(), one_gather=False,
              n_queues=1, sbuf_gather=True):
    wT, offw, offb = s["wT"], s["offw"], s["offb"]
    base_y, base_x = s["base_y"], s["base_x"]
    id16, id32 = s["id16"], s["id32"]
    x16p, xT_t, out_t, dbg = s["x16p"], s["xT_t"], s["out_t"], s["dbg"]
    xT_sb, ones1, xT3_t = s["xT_sb"], s["ones1"], s["xT3_t"]

    with ExitStack() as ctx:
        bpool = ctx.enter_context(tc.tile_pool(name="phaseB", bufs=1))
        if dbg:
            xtp = ctx.enter_context(tc.tile_pool(name="xtstage", bufs=3))
            xtv = xtp.tile([128, 256], F16, tag="xtv")
            for qt in range(25):
                q0 = min(qt * 128, HWp - 128)
                nc.sync.dma_start(xtv[:], xT_t.ap()[q0:q0 + 128, :])
                nc.sync.dma_start(dbg["xT"].ap()[q0:q0 + 128, :], xtv[:])

        if stop_after < 2:
            return
        # ---- phase C: offset conv ---------------------------------------
        convout = bpool.tile([18, 30 * W], F32)
        with tc.tile_pool(name="convps", bufs=2, space="PSUM") as cps:
            ntiles = [(0, 8), (8, 8), (16, 8), (24, 6)]
            for (row0, nrows) in ntiles:
                n = nrows * W
                ps = cps.tile([18, 8 * W], F32, tag="convps")
                first = True
                for tap in range(KK):
                    dy, dx = tap // 3, tap % 3
                    for ch in range(2):
                        rhs = x16p[:, ch, row0 + dy:row0 + dy + nrows, dx:dx + W]
                        nc.tensor.matmul(
                            ps[:, 0:n], offw[:, tap, ch, :], rhs,
                            start=first, stop=(tap == KK - 1 and ch == 1))
                        first = False
                nc.vector.tensor_scalar_add(convout[:, row0 * W:row0 * W + n], ps[:, 0:n], offb[:])
        if dbg:
            nc.sync.dma_start(dbg["conv"].ap(), convout[:, 0:NPOS])

        if stop_after < 3:
            return
        # ---- phase D: transpose offsets + prep --------------------------
        convT = bpool.tile([128, T, 18], F32)
        with tc.tile_pool(name="prepps", bufs=2, space="PSUM") as dps:
            for t in range(T):
                ps = dps.tile([128, 18], F32, tag="prepps")
                nc.tensor.transpose(ps[:], convout[:, t * 128:(t + 1) * 128], id32[:])
                nc.vector.tensor_copy(convT[:, t, :], ps[:])

        NF = T * KK  # 117
        pr = {k: bpool.tile([128, T, KK], F32, name=f"pr_{k}", tag=f"pr_{k}") for k in
              ("py", "px", "rn", "t0", "t1", "fy", "fx", "yc0", "yc1",
               "xb", "wa", "wb", "i2", "i3")}
        w4 = bpool.tile([128, T, 4, KK], F32)
        idxb = bpool.tile([128, T, KK], I16)

        def V(tl):  # full [128, NF] view
            return tl[:]

        # 1.5*2^23: x + MAGIC stays in [2^23, 2^24) where fp32 spacing is
        # exactly 1.0, so add-then-subtract rounds x to nearest integer even
        # for negative x (a bare 2^23 breaks below zero).
        TWO23 = float(3 * 2 ** 22)

        def floor_frac(src_off, base, py, y0_out, f_out, tmp0, tmp1):
            # py = conv offsets (stride-2 slice) + base; y0 = floor(py); f = frac
            nc.vector.tensor_tensor(V(py), convT[:, :, src_off::2], V(base), op=OP.add)
            nc.vector.tensor_scalar(V(tmp0), V(py), TWO23, TWO23, op0=OP.add, op1=OP.subtract)
            nc.vector.tensor_tensor(V(tmp1), V(tmp0), V(py), op=OP.is_gt)
            nc.vector.tensor_tensor(V(y0_out), V(tmp0), V(tmp1), op=OP.subtract)
            nc.vector.tensor_tensor(V(f_out), V(py), V(y0_out), op=OP.subtract)

        y0 = pr["rn"]; x0 = pr["t0"]
        floor_frac(0, base_y, pr["py"], y0, pr["fy"], pr["yc0"], pr["yc1"])
        floor_frac(1, base_x, pr["px"], x0, pr["fx"], pr["yc0"], pr["yc1"])

        # y side (pair gather): rows fetched are (yb, yb+1), yb=clip(y0,0,54).
        # BY0 = (1-fy)*J1 + fy*J2, BY1 = (1-fy)*J3 + fy*J1 where
        # J1=[y0 in 0..54], J2=[y0==-1], J3=[y0==55]  (mirrors the x algebra)
        nc.vector.tensor_scalar(V(pr["yc0"]), V(y0), 0.0, 54.0, op0=OP.max, op1=OP.min)  # yb
        nc.vector.tensor_tensor(V(pr["wa"]), V(y0), V(pr["yc0"]), op=OP.is_equal)   # J1
        nc.vector.tensor_scalar(V(pr["yc1"]), V(y0), -1.0, None, op0=OP.is_equal)   # J2
        nc.vector.tensor_scalar(V(y0), V(y0), 55.0, None, op0=OP.is_equal)          # J3 (in-place)
        nc.vector.tensor_scalar(V(pr["t1"]), V(pr["fy"]), 1.0, -1.0, op0=OP.subtract, op1=OP.mult)  # 1-fy
        by0 = pr["wa"]; by1 = pr["wb"]
        nc.vector.tensor_tensor(V(by1), V(pr["t1"]), V(y0), op=OP.mult)             # (1-fy)*J3
        nc.vector.tensor_tensor(V(pr["i2"]), V(pr["fy"]), V(pr["wa"]), op=OP.mult)  # fy*J1
        nc.vector.tensor_tensor(V(by1), V(by1), V(pr["i2"]), op=OP.add)             # BY1
        nc.vector.tensor_tensor(V(pr["i2"]), V(pr["t1"]), V(pr["wa"]), op=OP.mult)  # (1-fy)*J1
        nc.vector.tensor_tensor(V(pr["i3"]), V(pr["fy"]), V(pr["yc1"]), op=OP.mult) # fy*J2
        nc.vector.tensor_tensor(V(by0), V(pr["i2"]), V(pr["i3"]), op=OP.add)        # BY0

        # x side
        nc.vector.tensor_scalar(V(pr["xb"]), V(x0), 0.0, 54.0, op0=OP.max, op1=OP.min)
        nc.vector.tensor_tensor(V(pr["i2"]), V(x0), V(pr["xb"]), op=OP.is_equal)    # I1: x0 in [0,54]
        nc.vector.tensor_scalar(V(pr["i3"]), V(x0), -1.0, None, op0=OP.is_equal)    # I2: x0 == -1
        nc.vector.tensor_scalar(V(pr["t1"]), V(pr["fx"]), 1.0, -1.0, op0=OP.subtract, op1=OP.mult)  # 1-fx
        # ax0 = (1-fx)*I1 + fx*I2
        ax0 = pr["py"]; ax1 = pr["px"]  # reuse
        nc.vector.tensor_tensor(V(ax0), V(pr["t1"]), V(pr["i2"]), op=OP.mult)
        nc.vector.tensor_tensor(V(pr["i3"]), V(pr["fx"]), V(pr["i3"]), op=OP.mult)
        nc.vector.tensor_tensor(V(ax0), V(ax0), V(pr["i3"]), op=OP.add)
        # ax1 = (1-fx)*I3 + fx*I1
        nc.vector.tensor_scalar(V(pr["i3"]), V(x0), 55.0, None, op0=OP.is_equal)    # I3
        nc.vector.tensor_tensor(V(ax1), V(pr["t1"]), V(pr["i3"]), op=OP.mult)
        nc.vector.tensor_tensor(V(pr["i2"]), V(pr["fx"]), V(pr["i2"]), op=OP.mult)
        nc.vector.tensor_tensor(V(ax1), V(ax1), V(pr["i2"]), op=OP.add)
        # w4 slots (ys, xs)
        nc.vector.tensor_tensor(w4[:, :, 0, :], V(by0), V(ax0), op=OP.mult)
        nc.vector.tensor_tensor(w4[:, :, 1, :], V(by0), V(ax1), op=OP.mult)
        nc.vector.tensor_tensor(w4[:, :, 2, :], V(by1), V(ax0), op=OP.mult)
        nc.vector.tensor_tensor(w4[:, :, 3, :], V(by1), V(ax1), op=OP.mult)
        # pair index: idx = floor(yb/2)*56 + xb + (yb odd)*1568  (exact ints)
        nc.vector.tensor_scalar(V(pr["t1"]), V(pr["yc0"]), 0.5, -0.25, op0=OP.mult, op1=OP.add)
        nc.vector.tensor_scalar(V(pr["t1"]), V(pr["t1"]), TWO23, TWO23, op0=OP.add, op1=OP.subtract)  # yhf
        nc.vector.tensor_scalar(V(pr["i2"]), V(pr["t1"]), -2.0, None, op0=OP.mult)
        nc.vector.tensor_tensor(V(pr["i2"]), V(pr["i2"]), V(pr["yc0"]), op=OP.add)  # parity
        nc.vector.tensor_scalar(V(pr["t1"]), V(pr["t1"]), 56.0, None, op0=OP.mult)
        nc.vector.tensor_tensor(V(pr["t1"]), V(pr["t1"]), V(pr["xb"]), op=OP.add)
        nc.vector.tensor_scalar(V(pr["i2"]), V(pr["i2"]), 1568.0, None, op0=OP.mult)
        nc.vector.tensor_tensor(V(pr["t1"]), V(pr["t1"]), V(pr["i2"]), op=OP.add)
        nc.vector.tensor_copy(idxb[:, :, :], V(pr["t1"]))
        if dbg:
            nc.sync.dma_start(dbg["w4"].ap(), w4[:])

        # wrapped idx layout: [p%16, t, u=k, p//16]
        wrapped = bpool.tile([128, T, KK, 8], I16)
        for pg in range(8):
            nc.sync.dma_start(wrapped[0:16, :, :, pg], idxb[16 * pg:16 * (pg + 1), :, :])
        for half in (16, 32, 64):
            nc.sync.dma_start(wrapped[half:2 * half, :, :, :], wrapped[0:half, :, :, :])
        if dbg:
            nc.sync.dma_start(dbg["wrapped"].ap(), wrapped[:])

        if stop_after < 4:
            return
        # ---- phase E: gather / combine / transpose / GEMM ---------------
        gpool = ctx.enter_context(tc.tile_pool(name="gather", bufs=3))
        spool = ctx.enter_context(tc.tile_pool(name="sampled", bufs=2))
        tps = ctx.enter_context(tc.tile_pool(name="transps", bufs=2, space="PSUM"))
        xpool = ctx.enter_context(tc.tile_pool(name="xt", bufs=2))
        ops = ctx.enter_context(tc.tile_pool(name="outps", bufs=2, space="PSUM"))
        opool = ctx.enter_context(tc.tile_pool(name="outsb", bufs=2))
        wrp = ctx.enter_context(tc.tile_pool(name="w4rep", bufs=2))
        wps = ctx.enter_context(tc.tile_pool(name="w4ps", bufs=2, space="PSUM"))
        bps = ctx.enter_context(tc.tile_pool(name="bcastps", bufs=2, space="PSUM"))

        xT3_full = xT3_t.ap()
        src_ap = AP(tensor=xT3_full.tensor, offset=xT3_full.offset,
                    ap=[[512, 3136], [1, 1024]])

        # fp16 copy of w4 for the combine
        w4f = bpool.tile([128, T, 4, KK], F16)
        nc.vector.tensor_copy(w4f[:], w4[:])

        def g_view(g, ys, xs):
            gv = g[:]
            return AP(tensor=gv.tensor, offset=gv.offset + ys * 256 + xs * 512,
                      ap=[gv.ap[0], [1024, KK], [1, 256]])

        def w_view(t, slot):
            wv = w4f[:]
            return AP(tensor=wv.tensor, offset=wv.offset + (t * 4 + slot) * KK,
                      ap=[wv.ap[0], [1, KK], [0, 256]])

        Xt = None
        for t in range(T):
            ti = t % 4
            if ti == 0:
                Xt = xpool.tile([128, 18, 512], F16, tag="Xt")
            if sbuf_gather:
                # --- SBUF-source transposing gather: channel-major out ----
                # Xg[c_lane, blk=(xs*2+cblk), u=(k*2+ys), p]
                Xg = gpool.tile([128, 4, 18, 128], F16, tag="Xg")
                xgv = Xg[:]
                xg_out = AP(tensor=xgv.tensor, offset=xgv.offset,
                            ap=[xgv.ap[0], [2304, 4], [1, 2304]])
                if "gather" not in skip:
                    nc.gpsimd.dma_gather(
                        out_ap=xg_out, in_ap=xT_sb[:],
                        idxs_ap=wrapped[:, t, :, :],
                        num_idxs=2304, num_idxs_reg=2304, elem_size=512,
                        transpose=True, single_packet=False,
                        queue_num=t % n_queues,
                        sbuf_tokens_per_rank=64, sbuf_free_dim_per_rank=512)
                else:
                    nc.vector.memset(Xg[:, 0, 0, 0:16], 0.0)
                if dbg and t == 0:
                    nc.sync.dma_start(dbg["xg0"].ap(), Xg[:])
                if stop_after < 5:
                    continue
                # --- replicate w4[:, t] across c-partitions via PE --------
                # w4f[:, t] [128p, 36sk] --PE-T--> psum [36, 128p] --ACT-->
                # sbuf w4Ts; 36 rank-1 matmuls (ones x row) -> W4r [128, 36, 128]
                wpt = wps.tile([36, 128], F16, tag="w4ps")
                nc.tensor.transpose(wpt[:], w4f[:, t], id16[:])
                w4Ts = spool.tile([36, 128], F16, tag="w4Ts")
                nc.scalar.copy(w4Ts[:], wpt[:])
                # flatten [36,128] -> [1,4608] (partition->free), then 9
                # rank-1 outer products (ones x 512-slice) -> W4r replicated
                w4flat = spool.tile([1, 36 * 128], F16, tag="w4flat")
                nc.sync.dma_start(w4flat[:], w4Ts[:])
                W4r = wrp.tile([128, 36, 128], F16, tag="W4r")
                for j0 in range(0, 36, 4):
                    pb = bps.tile([128, 4, 128], F32, tag="bcast")
                    nc.tensor.matmul(
                        pb[:], ones1[:],
                        w4flat[:, j0 * 128:(j0 + 4) * 128],
                        start=True, stop=True)
                    nc.scalar.copy(W4r[:, j0:j0 + 4, :], pb[:])

                # --- channel-major bilinear combine directly into Xt ------
                wrv = W4r[:]

                def Wv(slot):
                    return AP(tensor=wrv.tensor,
                              offset=wrv.offset + slot * KK * 128,
                              ap=[wrv.ap[0], [0, 2], [128, KK], [1, 128]])

                def Cv(ys, xs):
                    return Xg[:, 2 * xs:2 * xs + 2, ys::2, :]

                xtv = Xt[:]
                xt_out = AP(tensor=xtv.tensor, offset=xtv.offset + ti * 128,
                            ap=[xtv.ap[0], [512, 2], [1024, KK], [1, 128]])
                if "combine" not in skip:
                    cA = spool.tile([128, 2, KK, 128], F16, tag="cA")
                    cB = spool.tile([128, 2, KK, 128], F16, tag="cB")
                    nc.vector.tensor_tensor(cA[:], Cv(0, 0), Wv(0), op=OP.mult)
                    nc.vector.tensor_tensor(cB[:], Cv(0, 1), Wv(1), op=OP.mult)
                    nc.vector.tensor_tensor(cA[:], cA[:], cB[:], op=OP.add)
                    nc.vector.tensor_tensor(cB[:], Cv(1, 0), Wv(2), op=OP.mult)
                    nc.vector.tensor_tensor(cA[:], cA[:], cB[:], op=OP.add)
                    nc.vector.tensor_tensor(cB[:], Cv(1, 1), Wv(3), op=OP.mult)
                    nc.vector.tensor_tensor(xt_out, cA[:], cB[:], op=OP.add)
                    if dbg and t == 0:
                        nc.sync.dma_start(dbg["w4r0"].ap(), W4r[:])
                        nc.sync.dma_start(dbg["w4ts0"].ap(), w4Ts[:])
                        nc.sync.dma_start(dbg["w4flat0"].ap(), w4flat[:])
                    if dbg and t == 3:
                        nc.sync.dma_start(dbg["xt0"].ap(), Xt[:])
                else:
                    nc.vector.memset(Xt[:, 0, ti * 128:ti * 128 + 16], 0.0)
                if stop_after < 6:
                    continue
            else:
                g = gpool.tile([128, KK, 1024], F16, tag="g")
                # pair gather: one 2KB elem = all 4 bilinear corners of a tap;
                # one multi-packet call per tile, queues rotated across tiles
                if "gather" not in skip:
                    nc.gpsimd.dma_gather(
                        out_ap=g[:], in_ap=src_ap,
                        idxs_ap=wrapped[:, t, :, :],
                        num_idxs=KK * 128, num_idxs_reg=KK * 128,
                        elem_size=1024, elem_step=512, single_packet=False,
                        queue_num=t % n_queues)
                else:
                    nc.vector.memset(g[:, 0, 0:16], 0.0)
                if dbg and t == 0:
                    nc.sync.dma_start(dbg["g0"].ap(), g[:])
                if stop_after < 5:
                    continue
                # 4 slot products on DVE; the bilinear add tree happens on
                # the PE: 4 accumulated transposes per chunk into PSUM
                ms = [spool.tile([128, KK, 256], F16, name=f"m{si}",
                                 tag=f"m{si}") for si in range(4)]
                if "combine" in skip:
                    for si in range(4):
                        nc.vector.memset(ms[si][:, 0, 0:16], 0.0)
                else:
                    nc.vector.tensor_tensor(ms[0][:], g_view(g, 0, 0), w_view(t, 0), op=OP.mult)
                    nc.vector.tensor_tensor(ms[1][:], g_view(g, 0, 1), w_view(t, 1), op=OP.mult)
                    nc.vector.tensor_tensor(ms[2][:], g_view(g, 1, 0), w_view(t, 2), op=OP.mult)
                    nc.vector.tensor_tensor(ms[3][:], g_view(g, 1, 1), w_view(t, 3), op=OP.mult)
                if stop_after < 6:
                    continue
                if "transpose" not in skip:
                    for q0 in range(0, 18, 4):
                        nq = min(4, 18 - q0)
                        pt = tps.tile([128, 4, 128], F32, tag="tps")
                        for qi in range(nq):
                            ct = q0 + qi
                            k, ch = ct // 2, ct % 2
                            for si in range(4):
                                nc.tensor.matmul(
                                    pt[:, qi, :],
                                    ms[si][:, k, ch * 128:(ch + 1) * 128],
                                    id16[:],
                                    start=(si == 0), stop=(si == 3))
                        nc.scalar.copy(
                            Xt[:, q0:q0 + nq, ti * 128:ti * 128 + 128],
                            pt[:, 0:nq, :])
                else:
                    nc.vector.memset(Xt[:, 0, ti * 128:ti * 128 + 16], 0.0)

            if "gemm" not in skip and (ti == 3 or t == T - 1):
                ncols = (ti + 1) * 128
                g0 = (t // 4) * 512
                for om in range(2):
                    pso = ops.tile([128, 512], F32, tag="outps")
                    for ct in range(18):
                        nc.tensor.matmul(
                            pso[:, 0:ncols], wT[:, ct, om * 128:(om + 1) * 128],
                            Xt[:, ct, 0:ncols],
                            start=(ct == 0), stop=(ct == 17))
                    osb = opool.tile([128, 512], F32, tag="outsb")
                    nc.scalar.copy(osb[:, 0:ncols], pso[:, 0:ncols])
                    nc.sync.dma_start(out_t.ap()[om, :, g0:g0 + ncols], osb[:, 0:ncols])

# ---------------------------------------------------------------------------
# host side
# ---------------------------------------------------------------------------
_CACHE = {}


def _get_program(reps=1, debug=False):
    key = (reps, debug)
    if key not in _CACHE:
        _CACHE[key] = build_program(reps, debug)
    return _CACHE[key]


def pack_inputs(x, weight, off_w, off_b):
    """Returns list of 8 per-core input dicts."""
    x = np.asarray(x, np.float32)
    weight = np.asarray(weight, np.float32)
    off_w = np.asarray(off_w, np.float32)
    off_b = np.asarray(off_b, np.float32)

    wr = weight.reshape(COUT, CIN, KK)
    wT = np.zeros((128, 18, 256), np.float16)
    for k in range(KK):
        for ch in range(2):
            # lhsT[c, o] = weight[o, ch*128+c, k]
            wT[:, k * 2 + ch, :] = wr[:, ch * 128:(ch + 1) * 128, k].T.astype(np.float16)
    owr = off_w.reshape(18, CIN, KK)
    offwT = np.zeros((128, KK, 2, 18), np.float16)
    for tap in range(KK):
        for ch in range(2):
            offwT[:, tap, ch, :] = owr[:, ch * 128:(ch + 1) * 128, tap].T.astype(np.float16)
    offb = off_b.reshape(18, 1).astype(np.float32)
    id16 = np.eye(128, dtype=np.float16)
    id32 = np.eye(18, dtype=np.float32)
    ones1v = np.ones((1, 128), np.float16)

    ky = (np.arange(KK) // 3).astype(np.float32)
    kx = (np.arange(KK) % 3).astype(np.float32)

    # position-major fp16 x per batch: xT[i, :] = x[b, :, i], zero rows >= HWp
    xTs, xT3s = [], []
    for b in range(B):
        xTb = np.zeros((HWp + 128, 256), np.float16)
        xTb[:HWp, :] = x[b].reshape(256, HWp).T.astype(np.float16)
        xTs.append(xTb)
        # pair-major copies: A[Y*56+x] = rows (2Y,2Y+1); B[Y*56+x] = (2Y+1,2Y+2)
        xv = xTb[:HWp]
        xT3 = np.zeros((3200, 512), np.float16)
        xT3[0:1568] = xv.reshape(28, 2, 56, 256).transpose(0, 2, 1, 3).reshape(1568, 512)
        xT3[1568:3080] = xv[56:56 + 54 * 56].reshape(27, 2, 56, 256).transpose(0, 2, 1, 3).reshape(1512, 512)
        xT3s.append(xT3)

    ins = []
    for core in range(8):
        b, half = core // 2, core % 2
        r0 = half * ROWS_HALF
        # conv window rows r0-1 .. r0+30 (32 rows), zero-padded outside [0,56),
        # width padded to 58 (zero cols 0 and 57), fp16
        x16p = np.zeros((2, 128, CONV_ROWS, PADW), np.float16)
        lo, hi = r0 - 1, r0 + 31
        slo, shi = max(lo, 0), min(hi, H)
        x16p[:, :, slo - lo:slo - lo + (shi - slo), 1:57] = \
            x[b].reshape(2, 128, H, W)[:, :, slo:shi, :].astype(np.float16)
        p_idx = np.arange(NPOS).reshape(T, 128).T.astype(np.float32)  # [128, T]
        ygrid = r0 + p_idx // W
        xgrid = p_idx % W
        base_y = (ygrid[:, :, None] - 1 + ky[None, None, :]).astype(np.float32)
        base_x = (xgrid[:, :, None] - 1 + kx[None, None, :]).astype(np.float32)
        ins.append({
            "xT": xTs[b], "xT3": xT3s[b],
            "x16p": x16p,
            "wT": wT, "offwT": offwT, "offb": offb,
            "base_y": np.ascontiguousarray(base_y),
            "base_x": np.ascontiguousarray(base_x),
            "ident16": id16, "ident32": id32, "ones1": ones1v,
        })
    return ins


def assemble_output(results):
    out = np.zeros((B, COUT, H, W), np.float32)
    for core in range(8):
        b, half = core // 2, core % 2
        r0 = half * ROWS_HALF
        o = results[core]["out"].reshape(COUT, NPOS)[:, :ROWS_HALF * W]
        out[b, :, r0:r0 + ROWS_HALF, :] = o.reshape(COUT, ROWS_HALF, W)
    return out


def kernel(x, weight, off_w, off_b):
    nc = _get_program(reps=1, debug=False)
    ins = pack_inputs(x, weight, off_w, off_b)
    res = run_bass_kernel_spmd(nc, ins, core_ids=list(range(8)))
    return assemble_output(res.results)



# revision 36
# speedup vs baseline: 1.0065x; 1.0065x over previous
# Deformable conv (B=4, C=256, 56x56, 3x3, COUT=256) on 8 Trainium2 cores.
#
# Sharding: core = b*2 + half; each core handles batch b, output rows
# [half*28, half*28+28). Data path in fp16; accumulation in fp32 PSUM;
# offsets/bilinear weights computed in fp32 on the DVE.
#
# Per-core pipeline (~200us/iter on HW):
#   C. offset conv as 9-tap implicit GEMM (fp16) -> offsets [18, 1664]
#   D. PE-transpose offsets to [128p, 13t, 18]; floor/frac via the 1.5*2^23
#      magic-round; border algebra folded into 4 bilinear slot weights
#      W4 [128p, 13t, 4slot, 9k]; ONE pair-gather index per (pos, tap):
#      idx = floor(yb/2)*56 + xb + (yb odd)*1568 into the pair-major table
#      (int16, 16-partition wrapped + tree-replicated for SWDGE)
#   E. per 128-position tile: 2 SWDGE dma_gathers (512/640 idx, on separate
#      SWDGE queues) of 2KB elems from the pair-major x^T copy in DRAM --
#      one elem = all 4 bilinear corners (rows yb,yb+1 x cols xb,xb+1) of a
#      tap; DVE computes only the 4 slot products [128p, 9k, 256c]; the
#      bilinear add tree runs on the PE as 4 PSUM-accumulated transpose
#      matmuls per 128-chunk (fp32 adds for free), ACT evicts PSUM->SBUF
#      f16; per 4 tiles: implicit GEMM over 18 chunks of 128 -> out
#      [256, 512] fp32 -> DRAM.
#
# Host packs: xT3 pair-major table (A copy = row pairs (2Y,2Y+1), B copy =
# (2Y+1,2Y+2)) so any clipped pair start yb in [0,54] is one 2KB elem;
# 3 SWDGE queues + 48KB dynamic DMA scratch keep gather drains overlapped.
import numpy as np
from contextlib import ExitStack

import concourse.bass as bass
import concourse.tile as tile
from concourse import bacc, mybir
from concourse.bass_types import AP
from concourse.bass_utils import run_bass_kernel_spmd

F32 = mybir.dt.float32
F16 = mybir.dt.float16
I16 = mybir.dt.int16
OP = mybir.AluOpType

B, CIN, H, W = 4, 256, 56, 56
COUT, KK = 256, 9
HWp = H * W            # 3136
NPOS = 1664            # 13 * 128 padded positions per core
T = 13                 # position tiles
ROWS_HALF = 28
CONV_ROWS = 32         # host-padded y window rows for conv input
PADW = 58              # x-padded width
CONV_FREE = CONV_ROWS * PADW  # 1856


def build_program(reps: int = 1, debug: bool = False, stop_after: int = 99,
                  skip: frozenset = frozenset(), one_gather: bool = False,
                  n_queues: int = 3, sbuf_gather: bool = False,
                  dma_scratch: int = 49152):
    nc = bacc.Bacc("TRN2", target_bir_lowering=False, debug=False, num_devices=8,
                   num_swdge_queues=n_queues,
                   dynamic_dma_scratch_size=dma_scratch)

    # ---- I/O -------------------------------------------------------------
    # xT: position-major fp16 x (pre-transposed on host), rows HWp..HWp+127 zero
    xT_t = nc.dram_tensor("xT", [HWp + 128, 256], F16, kind="ExternalInput")
    # pair-major x^T: row v = 512 f16 = [ch256 @ row 2Y+par, ch256 @ row 2Y+1+par]
    # A-copy (even pair starts) rows 0..1567, B-copy (odd starts) 1568..3079
    xT3_t = nc.dram_tensor("xT3", [3200, 512], F16, kind="ExternalInput")
    # x16p: host-padded fp16 conv input window [ch, 128, rows, PADW]
    xp_t = nc.dram_tensor("x16p", [2, 128, CONV_ROWS, PADW], F16, kind="ExternalInput")
    wT_t = nc.dram_tensor("wT", [128, 18, 256], F16, kind="ExternalInput")
    ow_t = nc.dram_tensor("offwT", [128, KK, 2, 18], F16, kind="ExternalInput")
    ob_t = nc.dram_tensor("offb", [18, 1], F32, kind="ExternalInput")
    by_t = nc.dram_tensor("base_y", [128, T, KK], F32, kind="ExternalInput")
    bx_t = nc.dram_tensor("base_x", [128, T, KK], F32, kind="ExternalInput")
    id16_t = nc.dram_tensor("ident16", [128, 128], F16, kind="ExternalInput")
    ones1_t = nc.dram_tensor("ones1", [1, 128], F16, kind="ExternalInput")
    id32_t = nc.dram_tensor("ident32", [18, 18], F32, kind="ExternalInput")
    out_t = nc.dram_tensor("out", [2, 128, NPOS], F32, kind="ExternalOutput")
    dbg = {}
    if debug:
        dbg["conv"] = nc.dram_tensor("dbg_conv", [18, NPOS], F32, kind="ExternalOutput")
        dbg["w4"] = nc.dram_tensor("dbg_w4", [128, T, 4, KK], F32, kind="ExternalOutput")
        dbg["wrapped"] = nc.dram_tensor("dbg_wrapped", [128, T, KK, 8], I16, kind="ExternalOutput")
        dbg["g0"] = nc.dram_tensor("dbg_g0", [128, KK, 1024], F16, kind="ExternalOutput")
        dbg["samp0"] = nc.dram_tensor("dbg_samp0", [128, KK, 256], F16, kind="ExternalOutput")
        dbg["xT"] = nc.dram_tensor("dbg_xT", [HWp + 1, 256], F16, kind="ExternalOutput")
        dbg["xg0"] = nc.dram_tensor("dbg_xg0", [128, 4, 18, 128], F16, kind="ExternalOutput")
        dbg["w4r0"] = nc.dram_tensor("dbg_w4r0", [128, 36, 128], F16, kind="ExternalOutput")
        dbg["xt0"] = nc.dram_tensor("dbg_xt0", [128, 18, 512], F16, kind="ExternalOutput")
        dbg["w4ts0"] = nc.dram_tensor("dbg_w4ts0", [36, 128], F16, kind="ExternalOutput")
        dbg["w4flat0"] = nc.dram_tensor("dbg_w4flat0", [1, 36 * 128], F16, kind="ExternalOutput")

    with tile.TileContext(nc) as tc, ExitStack() as ctx:
        # ---- persistent pools -------------------------------------------
        cpool = ctx.enter_context(tc.tile_pool(name="consts", bufs=1))
        wT = cpool.tile([128, 18, 256], F16)
        nc.sync.dma_start(wT[:], wT_t.ap())
        offw = cpool.tile([128, KK, 2, 18], F16)
        nc.sync.dma_start(offw[:], ow_t.ap())
        offb = cpool.tile([18, 1], F32)
        nc.sync.dma_start(offb[:], ob_t.ap())
        base_y = cpool.tile([128, T, KK], F32)
        nc.sync.dma_start(base_y[:], by_t.ap())
        base_x = cpool.tile([128, T, KK], F32)
        nc.sync.dma_start(base_x[:], bx_t.ap())
        id16 = cpool.tile([128, 128], F16)
        nc.sync.dma_start(id16[:], id16_t.ap())
        id32 = cpool.tile([18, 18], F32)
        nc.sync.dma_start(id32[:], id32_t.ap())
        x16p = cpool.tile([128, 2, CONV_ROWS, PADW], F16)
        nc.sync.dma_start(x16p[:, 0], xp_t.ap()[0])
        nc.sync.dma_start(x16p[:, 1], xp_t.ap()[1])
        ones1 = cpool.tile([1, 128], F16)
        nc.sync.dma_start(ones1[:], ones1_t.ap())
        xT_sb = None
        if sbuf_gather:
            _load_xt_sb = True
        else:
            _load_xt_sb = False
        # SBUF-resident x^T for SBUF-source transposing gather:
        # partition p<64 rank r holds xT row r*64+p; partition 64+p holds
        # row r*64+p+1 (the +1 shifted copy), so one 1024B gather elem =
        # rows (v, v+1) via stripe-major addressing (tokens_per_rank=64).
        if _load_xt_sb:
            xT_sb = cpool.tile([128, 51, 256], F16)
            xT_ap = xT_t.ap()
            nc.sync.dma_start(
                xT_sb[0:64, :, :],
                AP(tensor=xT_ap.tensor, offset=xT_ap.offset,
                   ap=[[256, 64], [16384, 51], [1, 256]]))
            nc.sync.dma_start(
                xT_sb[64:128, 0:50, :],
                AP(tensor=xT_ap.tensor, offset=xT_ap.offset + 256,
                   ap=[[256, 64], [16384, 50], [1, 256]]))

        for _rep in range(reps):
            _one_pass(nc, tc, dict(
                wT=wT, offw=offw, offb=offb, base_y=base_y, base_x=base_x,
                id16=id16, id32=id32, x16p=x16p, xT_t=xT_t, xT3_t=xT3_t,
                xT_sb=xT_sb,
                ones1=ones1,
                out_t=out_t, dbg=dbg), stop_after=stop_after, skip=skip,
                one_gather=one_gather, n_queues=n_queues,
                sbuf_gather=sbuf_gather)

    nc.compile()
    return nc


def _one_pass(nc, tc, s, stop_after=99, skip=frozenset(), one_gather=False,
              n_queues=1, sbuf_gather=True):
    wT, offw, offb = s["wT"], s["offw"], s["offb"]
    base_y, base_x = s["base_y"], s["base_x"]
    id16, id32 = s["id16"], s["id32"]
    x16p, xT_t, out_t, dbg = s["x16p"], s["xT_t"], s["out_t"], s["dbg"]
    xT_sb, ones1, xT3_t = s["xT_sb"], s["ones1"], s["xT3_t"]

    with ExitStack() as ctx:
        bpool = ctx.enter_context(tc.tile_pool(name="phaseB", bufs=1))
        if dbg:
            xtp = ctx.enter_context(tc.tile_pool(name="xtstage", bufs=3))
            xtv = xtp.tile([128, 256], F16, tag="xtv")
            for qt in range(25):
                q0 = min(qt * 128, HWp - 128)
                nc.sync.dma_start(xtv[:], xT_t.ap()[q0:q0 + 128, :])
                nc.sync.dma_start(dbg["xT"].ap()[q0:q0 + 128, :], xtv[:])

        if stop_after < 2:
            return
        # ---- phase C: offset conv ---------------------------------------
        convout = bpool.tile([18, 30 * W], F32)
        with tc.tile_pool(name="convps", bufs=2, space="PSUM") as cps:
            ntiles = [(0, 8), (8, 8), (16, 8), (24, 6)]
            for (row0, nrows) in ntiles:
                n = nrows * W
                ps = cps.tile([18, 8 * W], F32, tag="convps")
                first = True
                for tap in range(KK):
                    dy, dx = tap // 3, tap % 3
                    for ch in range(2):
                        rhs = x16p[:, ch, row0 + dy:row0 + dy + nrows, dx:dx + W]
                        nc.tensor.matmul(
                            ps[:, 0:n], offw[:, tap, ch, :], rhs,
                            start=first, stop=(tap == KK - 1 and ch == 1))
                        first = False
                nc.vector.tensor_scalar_add(convout[:, row0 * W:row0 * W + n], ps[:, 0:n], offb[:])
        if dbg:
            nc.sync.dma_start(dbg["conv"].ap(), convout[:, 0:NPOS])

        if stop_after < 3:
            return
        # ---- phase D: transpose offsets + prep --------------------------
        convT = bpool.tile([128, T, 18], F32)
        with tc.tile_pool(name="prepps", bufs=2, space="PSUM") as dps:
            for t in range(T):
                ps = dps.tile([128, 18], F32, tag="prepps")
                nc.tensor.transpose(ps[:], convout[:, t * 128:(t + 1) * 128], id32[:])
                nc.vector.tensor_copy(convT[:, t, :], ps[:])

        NF = T * KK  # 117
        pr = {k: bpool.tile([128, T, KK], F32, name=f"pr_{k}", tag=f"pr_{k}") for k in
              ("py", "px", "rn", "t0", "t1", "fy", "fx", "yc0", "yc1",
               "xb", "wa", "wb", "i2", "i3")}
        w4 = bpool.tile([128, T, 4, KK], F32)
        idxb = bpool.tile([128, T, KK], I16)

        def V(tl):  # full [128, NF] view
            return tl[:]

        # 1.5*2^23: x + MAGIC stays in [2^23, 2^24) where fp32 spacing is
        # exactly 1.0, so add-then-subtract rounds x to nearest integer even
        # for negative x (a bare 2^23 breaks below zero).
        TWO23 = float(3 * 2 ** 22)

        def floor_frac(src_off, base, py, y0_out, f_out, tmp0, tmp1):
            # py = conv offsets (stride-2 slice) + base; y0 = floor(py); f = frac
            nc.vector.tensor_tensor(V(py), convT[:, :, src_off::2], V(base), op=OP.add)
            nc.vector.tensor_scalar(V(tmp0), V(py), TWO23, TWO23, op0=OP.add, op1=OP.subtract)
            nc.vector.tensor_tensor(V(tmp1), V(tmp0), V(py), op=OP.is_gt)
            nc.vector.tensor_tensor(V(y0_out), V(tmp0), V(tmp1), op=OP.subtract)
            nc.vector.tensor_tensor(V(f_out), V(py), V(y0_out), op=OP.subtract)

        y0 = pr["rn"]; x0 = pr["t0"]
        floor_frac(0, base_y, pr["py"], y0, pr["fy"], pr["yc0"], pr["yc1"])
        floor_frac(1, base_x, pr["px"], x0, pr["fx"], pr["yc0"], pr["yc1"])

        # y side (pair gather): rows fetched are (yb, yb+1), yb=clip(y0,0,54).
        # BY0 = (1-fy)*J1 + fy*J2, BY1 = (1-fy)*J3 + fy*J1 where
        # J1=[y0 in 0..54], J2=[y0==-1], J3=[y0==55]  (mirrors the x algebra)
        nc.vector.tensor_scalar(V(pr["yc0"]), V(y0), 0.0, 54.0, op0=OP.max, op1=OP.min)  # yb
        nc.vector.tensor_tensor(V(pr["wa"]), V(y0), V(pr["yc0"]), op=OP.is_equal)   # J1
        nc.vector.tensor_scalar(V(pr["yc1"]), V(y0), -1.0, None, op0=OP.is_equal)   # J2
        nc.vector.tensor_scalar(V(y0), V(y0), 55.0, None, op0=OP.is_equal)          # J3 (in-place)
        nc.vector.tensor_scalar(V(pr["t1"]), V(pr["fy"]), 1.0, -1.0, op0=OP.subtract, op1=OP.mult)  # 1-fy
        by0 = pr["wa"]; by1 = pr["wb"]
        nc.vector.tensor_tensor(V(by1), V(pr["t1"]), V(y0), op=OP.mult)             # (1-fy)*J3
        nc.vector.tensor_tensor(V(pr["i2"]), V(pr["fy"]), V(pr["wa"]), op=OP.mult)  # fy*J1
        nc.vector.tensor_tensor(V(by1), V(by1), V(pr["i2"]), op=OP.add)             # BY1
        nc.vector.tensor_tensor(V(pr["i2"]), V(pr["t1"]), V(pr["wa"]), op=OP.mult)  # (1-fy)*J1
        nc.vector.tensor_tensor(V(pr["i3"]), V(pr["fy"]), V(pr["yc1"]), op=OP.mult) # fy*J2
        nc.vector.tensor_tensor(V(by0), V(pr["i2"]), V(pr["i3"]), op=OP.add)        # BY0

        # x side
        nc.vector.tensor_scalar(V(pr["xb"]), V(x0), 0.0, 54.0, op0=OP.max, op1=OP.min)
        nc.vector.tensor_tensor(V(pr["i2"]), V(x0), V(pr["xb"]), op=OP.is_equal)    # I1: x0 in [0,54]
        nc.vector.tensor_scalar(V(pr["i3"]), V(x0), -1.0, None, op0=OP.is_equal)    # I2: x0 == -1
        nc.vector.tensor_scalar(V(pr["t1"]), V(pr["fx"]), 1.0, -1.0, op0=OP.subtract, op1=OP.mult)  # 1-fx
        # ax0 = (1-fx)*I1 + fx*I2
        ax0 = pr["py"]; ax1 = pr["px"]  # reuse
        nc.vector.tensor_tensor(V(ax0), V(pr["t1"]), V(pr["i2"]), op=OP.mult)
        nc.vector.tensor_tensor(V(pr["i3"]), V(pr["fx"]), V(pr["i3"]), op=OP.mult)
        nc.vector.tensor_tensor(V(ax0), V(ax0), V(pr["i3"]), op=OP.add)
        # ax1 = (1-fx)*I3 + fx*I1
        nc.vector.tensor_scalar(V(pr["i3"]), V(x0), 55.0, None, op0=OP.is_equal)    # I3
        nc.vector.tensor_tensor(V(ax1), V(pr["t1"]), V(pr["i3"]), op=OP.mult)
        nc.vector.tensor_tensor(V(pr["i2"]), V(pr["fx"]), V(pr["i2"]), op=OP.mult)
        nc.vector.tensor_tensor(V(ax1), V(ax1), V(pr["i2"]), op=OP.add)
        # w4 slots (ys, xs)
        nc.vector.tensor_tensor(w4[:, :, 0, :], V(by0), V(ax0), op=OP.mult)
        nc.vector.tensor_tensor(w4[:, :, 1, :], V(by0), V(ax1), op=OP.mult)
        nc.vector.tensor_tensor(w4[:, :, 2, :], V(by1), V(ax0), op=OP.mult)
        nc.vector.tensor_tensor(w4[:, :, 3, :], V(by1), V(ax1), op=OP.mult)
        # pair index: idx = floor(yb/2)*56 + xb + (yb odd)*1568  (exact ints)
        nc.vector.tensor_scalar(V(pr["t1"]), V(pr["yc0"]), 0.5, -0.25, op0=OP.mult, op1=OP.add)
        nc.vector.tensor_scalar(V(pr["t1"]), V(pr["t1"]), TWO23, TWO23, op0=OP.add, op1=OP.subtract)  # yhf
        nc.vector.tensor_scalar(V(pr["i2"]), V(pr["t1"]), -2.0, None, op0=OP.mult)
        nc.vector.tensor_tensor(V(pr["i2"]), V(pr["i2"]), V(pr["yc0"]), op=OP.add)  # parity
        nc.vector.tensor_scalar(V(pr["t1"]), V(pr["t1"]), 56.0, None, op0=OP.mult)
        nc.vector.tensor_tensor(V(pr["t1"]), V(pr["t1"]), V(pr["xb"]), op=OP.add)
        nc.vector.tensor_scalar(V(pr["i2"]), V(pr["i2"]), 1568.0, None, op0=OP.mult)
        nc.vector.tensor_tensor(V(pr["t1"]), V(pr["t1"]), V(pr["i2"]), op=OP.add)
        nc.vector.tensor_copy(idxb[:, :, :], V(pr["t1"]))
        if dbg:
            nc.sync.dma_start(dbg["w4"].ap(), w4[:])

        # wrapped idx layout: [p%16, t, u=k, p//16]
        wrapped = bpool.tile([128, T, KK, 8], I16)
        for pg in range(8):
            nc.sync.dma_start(wrapped[0:16, :, :, pg], idxb[16 * pg:16 * (pg + 1), :, :])
        for half in (16, 32, 64):
            nc.sync.dma_start(wrapped[half:2 * half, :, :, :], wrapped[0:half, :, :, :])
        if dbg:
            nc.sync.dma_start(dbg["wrapped"].ap(), wrapped[:])

        if stop_after < 4:
            return
        # ---- phase E: gather / combine / transpose / GEMM ---------------
        gpool = ctx.enter_context(tc.tile_pool(name="gather", bufs=3))
        spool = ctx.enter_context(tc.tile_pool(name="sampled", bufs=2))
        tps = ctx.enter_context(tc.tile_pool(name="transps", bufs=2, space="PSUM"))
        xpool = ctx.enter_context(tc.tile_pool(name="xt", bufs=2))
        ops = ctx.enter_context(tc.tile_pool(name="outps", bufs=2, space="PSUM"))
        opool = ctx.enter_context(tc.tile_pool(name="outsb", bufs=2))
        wrp = ctx.enter_context(tc.tile_pool(name="w4rep", bufs=2))
        wps = ctx.enter_context(tc.tile_pool(name="w4ps", bufs=2, space="PSUM"))
        bps = ctx.enter_context(tc.tile_pool(name="bcastps", bufs=2, space="PSUM"))

        xT3_full = xT3_t.ap()
        src_ap = AP(tensor=xT3_full.tensor, offset=xT3_full.offset,
                    ap=[[512, 3136], [1, 1024]])

        # fp16 copy of w4 for the combine
        w4f = bpool.tile([128, T, 4, KK], F16)
        nc.vector.tensor_copy(w4f[:], w4[:])

        def g_view(g, ys, xs):
            gv = g[:]
            return AP(tensor=gv.tensor, offset=gv.offset + ys * 256 + xs * 512,
                      ap=[gv.ap[0], [1024, KK], [1, 256]])

        def w_view(t, slot):
            wv = w4f[:]
            return AP(tensor=wv.tensor, offset=wv.offset + (t * 4 + slot) * KK,
                      ap=[wv.ap[0], [1, KK], [0, 256]])

        Xt = None
        for t in range(T):
            ti = t % 4
            if ti == 0:
                Xt = xpool.tile([128, 18, 512], F16, tag="Xt")
            if sbuf_gather:
                # --- SBUF-source transposing gather: channel-major out ----
                # Xg[c_lane, blk=(xs*2+cblk), u=(k*2+ys), p]
                Xg = gpool.tile([128, 4, 18, 128], F16, tag="Xg")
                xgv = Xg[:]
                xg_out = AP(tensor=xgv.tensor, offset=xgv.offset,
                            ap=[xgv.ap[0], [2304, 4], [1, 2304]])
                if "gather" not in skip:
                    nc.gpsimd.dma_gather(
                        out_ap=xg_out, in_ap=xT_sb[:],
                        idxs_ap=wrapped[:, t, :, :],
                        num_idxs=2304, num_idxs_reg=2304, elem_size=512,
                        transpose=True, single_packet=False,
                        queue_num=t % n_queues,
                        sbuf_tokens_per_rank=64, sbuf_free_dim_per_rank=512)
                else:
                    nc.vector.memset(Xg[:, 0, 0, 0:16], 0.0)
                if dbg and t == 0:
                    nc.sync.dma_start(dbg["xg0"].ap(), Xg[:])
                if stop_after < 5:
                    continue
                # --- replicate w4[:, t] across c-partitions via PE --------
                # w4f[:, t] [128p, 36sk] --PE-T--> psum [36, 128p] --ACT-->
                # sbuf w4Ts; 36 rank-1 matmuls (ones x row) -> W4r [128, 36, 128]
                wpt = wps.tile([36, 128], F16, tag="w4ps")
                nc.tensor.transpose(wpt[:], w4f[:, t], id16[:])
                w4Ts = spool.tile([36, 128], F16, tag="w4Ts")
                nc.scalar.copy(w4Ts[:], wpt[:])
                # flatten [36,128] -> [1,4608] (partition->free), then 9
                # rank-1 outer products (ones x 512-slice) -> W4r replicated
                w4flat = spool.tile([1, 36 * 128], F16, tag="w4flat")
                nc.sync.dma_start(w4flat[:], w4Ts[:])
                W4r = wrp.tile([128, 36, 128], F16, tag="W4r")
                for j0 in range(0, 36, 4):
                    pb = bps.tile([128, 4, 128], F32, tag="bcast")
                    nc.tensor.matmul(
                        pb[:], ones1[:],
                        w4flat[:, j0 * 128:(j0 + 4) * 128],
                        start=True, stop=True)
                    nc.scalar.copy(W4r[:, j0:j0 + 4, :], pb[:])

                # --- channel-major bilinear combine directly into Xt ------
                wrv = W4r[:]

                def Wv(slot):
                    return AP(tensor=wrv.tensor,
                              offset=wrv.offset + slot * KK * 128,
                              ap=[wrv.ap[0], [0, 2], [128, KK], [1, 128]])

                def Cv(ys, xs):
                    return Xg[:, 2 * xs:2 * xs + 2, ys::2, :]

                xtv = Xt[:]
                xt_out = AP(tensor=xtv.tensor, offset=xtv.offset + ti * 128,
                            ap=[xtv.ap[0], [512, 2], [1024, KK], [1, 128]])
                if "combine" not in skip:
                    cA = spool.tile([128, 2, KK, 128], F16, tag="cA")
                    cB = spool.tile([128, 2, KK, 128], F16, tag="cB")
                    nc.vector.tensor_tensor(cA[:], Cv(0, 0), Wv(0), op=OP.mult)
                    nc.vector.tensor_tensor(cB[:], Cv(0, 1), Wv(1), op=OP.mult)
                    nc.vector.tensor_tensor(cA[:], cA[:], cB[:], op=OP.add)
                    nc.vector.tensor_tensor(cB[:], Cv(1, 0), Wv(2), op=OP.mult)
                    nc.vector.tensor_tensor(cA[:], cA[:], cB[:], op=OP.add)
                    nc.vector.tensor_tensor(cB[:], Cv(1, 1), Wv(3), op=OP.mult)
                    nc.vector.tensor_tensor(xt_out, cA[:], cB[:], op=OP.add)
                    if dbg and t == 0:
                        nc.sync.dma_start(dbg["w4r0"].ap(), W4r[:])
                        nc.sync.dma_start(dbg["w4ts0"].ap(), w4Ts[:])
                        nc.sync.dma_start(dbg["w4flat0"].ap(), w4flat[:])
                    if dbg and t == 3:
                        nc.sync.dma_start(dbg["xt0"].ap(), Xt[:])
                else:
                    nc.vector.memset(Xt[:, 0, ti * 128:ti * 128 + 16], 0.0)
                if stop_after < 6:
                    continue
            else:
                g = gpool.tile([128, KK, 1024], F16, tag="g")
                # pair gather: one 2KB elem = all 4 bilinear corners of a tap
                if "gather" not in skip:
                    for hu, (u0, nu) in enumerate(((0, 4), (4, 5))):
                        nidx = nu * 128
                        nc.gpsimd.dma_gather(
                            out_ap=g[:, u0:u0 + nu, :], in_ap=src_ap,
                            idxs_ap=wrapped[:, t, u0:u0 + nu, :],
                            num_idxs=nidx, num_idxs_reg=nidx, elem_size=1024,
                            elem_step=512, queue_num=hu % n_queues)
                else:
                    nc.vector.memset(g[:, 0, 0:16], 0.0)
                if dbg and t == 0:
                    nc.sync.dma_start(dbg["g0"].ap(), g[:])
                if stop_after < 5:
                    continue
                # 4 slot products on DVE; the bilinear add tree happens on
                # the PE: 4 accumulated transposes per chunk into PSUM
                ms = [spool.tile([128, KK, 256], F16, name=f"m{si}",
                                 tag=f"m{si}") for si in range(4)]
                if "combine" in skip:
                    for si in range(4):
                        nc.vector.memset(ms[si][:, 0, 0:16], 0.0)
                else:
                    nc.vector.tensor_tensor(ms[0][:], g_view(g, 0, 0), w_view(t, 0), op=OP.mult)
                    nc.vector.tensor_tensor(ms[1][:], g_view(g, 0, 1), w_view(t, 1), op=OP.mult)
                    nc.vector.tensor_tensor(ms[2][:], g_view(g, 1, 0), w_view(t, 2), op=OP.mult)
                    nc.vector.tensor_tensor(ms[3][:], g_view(g, 1, 1), w_view(t, 3), op=OP.mult)
                if stop_after < 6:
                    continue
                if "transpose" not in skip:
                    for q0 in range(0, 18, 4):
                        nq = min(4, 18 - q0)
                        pt = tps.tile([128, 4, 128], F32, tag="tps")
                        for qi in range(nq):
                            ct = q0 + qi
                            k, ch = ct // 2, ct % 2
                            for si in range(4):
                                nc.tensor.matmul(
                                    pt[:, qi, :],
                                    ms[si][:, k, ch * 128:(ch + 1) * 128],
                                    id16[:],
                                    start=(si == 0), stop=(si == 3))
                        nc.scalar.copy(
                            Xt[:, q0:q0 + nq, ti * 128:ti * 128 + 128],
                            pt[:, 0:nq, :])
                else:
                    nc.vector.memset(Xt[:, 0, ti * 128:ti * 128 + 16], 0.0)

            if "gemm" not in skip and (ti == 3 or t == T - 1):
                ncols = (ti + 1) * 128
                g0 = (t // 4) * 512
                for om in range(2):
                    pso = ops.tile([128, 512], F32, tag="outps")
                    for ct in range(18):
                        nc.tensor.matmul(
                            pso[:, 0:ncols], wT[:, ct, om * 128:(om + 1) * 128],
                            Xt[:, ct, 0:ncols],
                            start=(ct == 0), stop=(ct == 17))
                    osb = opool.tile([128, 512], F32, tag="outsb")
                    nc.scalar.copy(osb[:, 0:ncols], pso[:, 0:ncols])
                    nc.sync.dma_start(out_t.ap()[om, :, g0:g0 + ncols], osb[:, 0:ncols])

# ---------------------------------------------------------------------------
# host side
# ---------------------------------------------------------------------------
_CACHE = {}


def _get_program(reps=1, debug=False):
    key = (reps, debug)
    if key not in _CACHE:
        _CACHE[key] = build_program(reps, debug)
    return _CACHE[key]


def pack_inputs(x, weight, off_w, off_b):
    """Returns list of 8 per-core input dicts."""
    x = np.asarray(x, np.float32)
    weight = np.asarray(weight, np.float32)
    off_w = np.asarray(off_w, np.float32)
    off_b = np.asarray(off_b, np.float32)

    wr = weight.reshape(COUT, CIN, KK)
    wT = np.zeros((128, 18, 256), np.float16)
    for k in range(KK):
        for ch in range(2):
            # lhsT[c, o] = weight[o, ch*128+c, k]
            wT[:, k * 2 + ch, :] = wr[:, ch * 128:(ch + 1) * 128, k].T.astype(np.float16)
    owr = off_w.reshape(18, CIN, KK)
    offwT = np.zeros((128, KK, 2, 18), np.float16)
    for tap in range(KK):
        for ch in range(2):
            offwT[:, tap, ch, :] = owr[:, ch * 128:(ch + 1) * 128, tap].T.astype(np.float16)
    offb = off_b.reshape(18, 1).astype(np.float32)
    id16 = np.eye(128, dtype=np.float16)
    id32 = np.eye(18, dtype=np.float32)
    ones1v = np.ones((1, 128), np.float16)

    ky = (np.arange(KK) // 3).astype(np.float32)
    kx = (np.arange(KK) % 3).astype(np.float32)

    # position-major fp16 x per batch: xT[i, :] = x[b, :, i], zero rows >= HWp
    xTs, xT3s = [], []
    for b in range(B):
        xTb = np.zeros((HWp + 128, 256), np.float16)
        xTb[:HWp, :] = x[b].reshape(256, HWp).T.astype(np.float16)
        xTs.append(xTb)
        # pair-major copies: A[Y*56+x] = rows (2Y,2Y+1); B[Y*56+x] = (2Y+1,2Y+2)
        xv = xTb[:HWp]
        xT3 = np.zeros((3200, 512), np.float16)
        xT3[0:1568] = xv.reshape(28, 2, 56, 256).transpose(0, 2, 1, 3).reshape(1568, 512)
        xT3[1568:3080] = xv[56:56 + 54 * 56].reshape(27, 2, 56, 256).transpose(0, 2, 1, 3).reshape(1512, 512)
        xT3s.append(xT3)

    ins = []
    for core in range(8):
        b, half = core // 2, core % 2
        r0 = half * ROWS_HALF
        # conv window rows r0-1 .. r0+30 (32 rows), zero-padded outside [0,56),
        # width padded to 58 (zero cols 0 and 57), fp16
        x16p = np.zeros((2, 128, CONV_ROWS, PADW), np.float16)
        lo, hi = r0 - 1, r0 + 31
        slo, shi = max(lo, 0), min(hi, H)
        x16p[:, :, slo - lo:slo - lo + (shi - slo), 1:57] = \
            x[b].reshape(2, 128, H, W)[:, :, slo:shi, :].astype(np.float16)
        p_idx = np.arange(NPOS).reshape(T, 128).T.astype(np.float32)  # [128, T]
        ygrid = r0 + p_idx // W
        xgrid = p_idx % W
        base_y = (ygrid[:, :, None] - 1 + ky[None, None, :]).astype(np.float32)
        base_x = (xgrid[:, :, None] - 1 + kx[None, None, :]).astype(np.float32)
        ins.append({
            "xT": xTs[b], "xT3": xT3s[b],
            "x16p": x16p,
            "wT": wT, "offwT": offwT, "offb": offb,
            "base_y": np.ascontiguousarray(base_y),
            "base_x": np.ascontiguousarray(base_x),
            "ident16": id16, "ident32": id32, "ones1": ones1v,
        })
    return ins


def assemble_output(results):
    out = np.zeros((B, COUT, H, W), np.float32)
    for core in range(8):
        b, half = core // 2, core % 2
        r0 = half * ROWS_HALF
        o = results[core]["out"].reshape(COUT, NPOS)[:, :ROWS_HALF * W]
        out[b, :, r0:r0 + ROWS_HALF, :] = o.reshape(COUT, ROWS_HALF, W)
    return out


def kernel(x, weight, off_w, off_b):
    nc = _get_program(reps=1, debug=False)
    ins = pack_inputs(x, weight, off_w, off_b)
    res = run_bass_kernel_spmd(nc, ins, core_ids=list(range(8)))
    return assemble_output(res.results)

